# revision 14
# baseline (speedup 1.0000x reference)
"""Trainium2 Bass kernel for nn_InvLocalPatOrientConvolution.

Computation:
  1. Host: synthesize the 160-channel 5x5x5 conv filter from
     weight/zeroweight/basis_functions/wigner indices (3.2 MFLOP) and lay out
     per-core operands (fp16).
  2. Device (8 NeuronCores, SPMD): VALID 3D conv as PE matmuls (contraction =
     5 y-taps x 16 input channels = 80 partitions; x/z tap shifts expressed as
     AP offsets into a 5x-replicated SBUF-resident input) + SO(3) grid pooling
     (relu-weighted second-moment ratio) also on the PE.
     Channel split: 128-channel A-tile (full array) + 32-channel B-tile packed
     4 chunks at a time into the 4 PE column groups via tile_position.
     Sharding: batch (2) x output-X-slabs (4) -> 8 cores.
  3. Host: gather per-core slabs into the full (2,16,36,36,36) output.
"""

import os
import sys

for _p in ("/root/.axon_site/_ro/trn_rl_repo", "/opt/trn_rl_repo"):
    if os.path.isdir(_p) and _p not in sys.path:
        sys.path.insert(0, _p)

import numpy as np

import concourse.mybir as mybir
from concourse import bacc
from concourse.tile import TileContext
from concourse.bass_utils import run_bass_kernel_spmd

# Problem constants (hardcoded per harness contract)
ORDER = 2
KS = 5            # conv kernel size
CIN = 16
COUT = 16
EPS = 1e-16
S = 10            # wigner rows
B = 2
D_IN = 40         # input spatial
D_OUT = 36        # output spatial
SLAB = 9          # output X planes per core (36/4)
SLAB_IN = SLAB + KS - 1   # 13 input X planes per core
NCORES = 8
YB = 12           # y-block per chunk
NCHUNK = YB * D_OUT       # 432 columns per matmul chunk

F16 = mybir.dt.float16
F32 = mybir.dt.float32

_prog_cache = {}


def _build_program(repeat=1):
    """Build the SPMD device program (identical on all 8 cores)."""
    nc = bacc.Bacc("TRN2")

    r_d = nc.dram_tensor("r", [80, SLAB_IN, D_OUT, D_IN], F16, kind="ExternalInput")
    w_d = nc.dram_tensor("w", [25, 80, 160], F16, kind="ExternalInput")
    ga_d = nc.dram_tensor("ga", [128, 4, 108], F16, kind="ExternalInput")
    gb_d = nc.dram_tensor("gb", [128, 108], F16, kind="ExternalInput")
    wnd_d = nc.dram_tensor("wnd", [108, 64], F16, kind="ExternalInput")
    bias_d = nc.dram_tensor("bias", [16, 1], F32, kind="ExternalInput")
    y_d = nc.dram_tensor("y", [16, SLAB, D_OUT, D_OUT], F32, kind="ExternalOutput")

    chunks = [(xr, cy) for xr in range(SLAB) for cy in range(3)]
    groups = [chunks[i:i + 4] for i in range(0, len(chunks), 4)]

    with TileContext(nc) as tc:
        with tc.tile_pool(name="const", bufs=1) as cpool, \
             tc.tile_pool(name="work", bufs=3) as wpool, \
             tc.tile_pool(name="casb", bufs=9) as capool, \
             tc.tile_pool(name="rrel", bufs=5) as rpool, \
             tc.tile_pool(name="conv_ps", bufs=4, space="PSUM") as conv_pool, \
             tc.tile_pool(name="convb_ps", bufs=1, space="PSUM") as convb_pool, \
             tc.tile_pool(name="a_ps", bufs=2, space="PSUM") as a_pool, \
             tc.tile_pool(name="nd_ps", bufs=1, space="PSUM") as nd_pool:

            # ---- resident constants (weights first so chunk 0 can start
            # as soon as its 5 input planes land) ----
            wt = cpool.tile([80, 25, 160], F16, tag="wt2")
            for ik in range(25):
                nc.sync.dma_start(out=wt[:, ik, :], in_=w_d[ik, :, :])
            gat = cpool.tile([128, 4, 108], F16)
            gbt = cpool.tile([128, 108], F16)
            wndt = cpool.tile([108, 64], F16)
            biast = cpool.tile([16, 1], F32)
            nc.sync.dma_start(out=gat[:], in_=ga_d[:])
            nc.sync.dma_start(out=gbt[:], in_=gb_d[:])
            nc.sync.dma_start(out=wndt[:], in_=wnd_d[:])
            nc.sync.dma_start(out=biast[:], in_=bias_d[:])
            rts = []
            for p in range(SLAB_IN):
                rt = cpool.tile([80, D_OUT, D_IN], F16, tag=f"rt{p}")
                nc.sync.dma_start(out=rt[:], in_=r_d[:, p, :, :])
                rts.append(rt)

            for _rep in range(repeat):
              for grp in groups:
                # ---- conv A-tiles (128 channels, full array) ----
                ca_sbs = []
                for (xr, cy) in grp:
                    y0 = cy * YB
                    cps = conv_pool.tile([128, NCHUNK], F32, tag="cps")
                    t = 0
                    for i in range(KS):
                        for k in range(KS):
                            rhs = rts[xr + i][:, y0:y0 + YB, k:k + D_OUT]
                            lhsT = wt[:, i * KS + k, 0:128]
                            nc.tensor.matmul(cps[:], lhsT, rhs,
                                             start=(t == 0), stop=(t == 24))
                            t += 1
                    ca = capool.tile([128, NCHUNK], F16, tag="ca")
                    nc.scalar.copy(ca[:], cps[:])
                    ca_sbs.append(ca)

                # ---- conv B-tile (32 channels) col-tiled over the group ----
                cbps = convb_pool.tile([128, NCHUNK], F32, tag="cbps")
                for t, (i, k) in enumerate((i, k) for i in range(KS)
                                           for k in range(KS)):
                    lhsT = wt[:, i * KS + k, 128:160]
                    for c, (xr, cy) in enumerate(grp):
                        y0 = cy * YB
                        rhs = rts[xr + i][:, y0:y0 + YB, k:k + D_OUT]
                        nc.tensor.matmul(
                            cbps[32 * c:32 * (c + 1), :], lhsT, rhs,
                            start=(t == 0), stop=(t == 24),
                            tile_position=(0, 32 * c),
                        )
                cb = capool.tile([128, NCHUNK], F16, tag="cb")
                nc.scalar.copy(cb[:], cbps[:])

                # ---- so3 pooling per chunk ----
                for c, (xr, cy) in enumerate(grp):
                    y0 = cy * YB
                    # num/den partial sums packed into the 4 PE column groups
                    # of ONE psum tile: rows 0-15 / 32-47 = num (mt even/odd),
                    # rows 64-79 / 96-111 = den (mt even/odd).
                    nd_ps = nd_pool.tile([128, NCHUNK], F32, tag="nd")
                    rrels, r2s = [], []
                    for mt in range(4):
                        aps = a_pool.tile([108, NCHUNK], F32, tag="aps")
                        last = (mt == 3)
                        nc.tensor.matmul(aps[:], gat[:, mt, :], ca_sbs[c][:],
                                         start=True, stop=not last)
                        if last:
                            nc.tensor.matmul(
                                aps[:],
                                gbt[32 * c:32 * (c + 1), :],
                                cb[32 * c:32 * (c + 1), :],
                                start=False, stop=True,
                                tile_position=(32 * c, 0),
                            )
                        rrel = rpool.tile([108, NCHUNK], F16, tag="rrel")
                        nc.scalar.activation(rrel[:], aps[:],
                                             mybir.ActivationFunctionType.Relu)
                        r2 = rpool.tile([108, NCHUNK], F16, tag="r2")
                        nc.vector.tensor_mul(r2[:], rrel[:], rrel[:])
                        rrels.append(rrel)
                        r2s.append(r2)
                    # all 8 partial-moment matmuls back-to-back so the 4 PE
                    # column groups stream them concurrently (span ~2N)
                    for mt in range(4):
                        wnd_g = wndt[:, mt * 16:(mt + 1) * 16]
                        cg = 32 * (mt % 2)
                        nc.tensor.matmul(nd_ps[cg:cg + 16, :], wnd_g,
                                         r2s[mt][:],
                                         start=(mt < 2), stop=(mt >= 2),
                                         tile_position=(0, cg))
                        nc.tensor.matmul(nd_ps[64 + cg:64 + cg + 16, :],
                                         wnd_g, rrels[mt][:],
                                         start=(mt < 2), stop=(mt >= 2),
                                         tile_position=(0, 64 + cg))

                    num_a = wpool.tile([16, NCHUNK], F32, tag="num_a")
                    nc.scalar.copy(num_a[:], nd_ps[0:16, :])
                    den_a = wpool.tile([16, NCHUNK], F32, tag="den_a")
                    nc.scalar.activation(den_a[:], nd_ps[64:80, :],
                                         mybir.ActivationFunctionType.Copy,
                                         bias=EPS)
                    num_sb = wpool.tile([16, NCHUNK], F32, tag="num_sb")
                    nc.vector.tensor_add(num_sb[:], num_a[:], nd_ps[32:48, :])
                    den_sb = wpool.tile([16, NCHUNK], F32, tag="den_sb")
                    nc.vector.tensor_add(den_sb[:], den_a[:], nd_ps[96:112, :])
                    recip = wpool.tile([16, NCHUNK], F32, tag="recip")
                    nc.vector.reciprocal(recip[:], den_sb[:])
                    out_sb = wpool.tile([16, NCHUNK], F32, tag="out_sb")
                    nc.vector.tensor_mul(out_sb[:], num_sb[:], recip[:])
                    nc.vector.tensor_scalar_add(out_sb[:], out_sb[:],
                                                biast[:, 0:1])

                    dst = y_d[:, xr, y0:y0 + YB, :]
                    nc.sync.dma_start(out=dst, in_=out_sb[:].rearrange(
                        "p (a b) -> p a b", a=YB))

    nc.finalize()
    return nc


def _synthesize_filter(weight, zeroweight, basis_functions, wig_w, wig_b):
    """Replicate the reference's kernel synthesis in fp32 numpy.

    Returns kern6[l, e, d, i, j, k] of shape (10, 16, 16, 5, 5, 5)."""
    zero_ext = np.concatenate(
        [zeroweight[None, None],
         np.zeros((ORDER ** 2 - 1, 1, CIN, COUT), weight.dtype)], axis=0)
    wfull = np.concatenate([zero_ext, weight], axis=1)       # (4, 10, 16, 16)
    wg = wfull[wig_w]                                        # (10, 10, 16, 16)
    bg = basis_functions[wig_b]                              # (10, 10, 5, 5, 5)
    kern6 = np.einsum("lred,lrijk->ledijk", wg, bg)          # (10,16,16,5,5,5)
    return np.ascontiguousarray(kern6.astype(np.float32))


def _host_prep(x, weight, zeroweight, bias, so3basisgrid, w_i,
               basis_functions, wig_w, wig_b):
    kern6 = _synthesize_filter(weight, zeroweight, basis_functions, wig_w, wig_b)

    # conv weights: W[i*5+k, j*16+d, e*10+l]  (cols e-major; A = cols 0..127)
    w_arr = np.ascontiguousarray(
        kern6.transpose(3, 5, 4, 2, 1, 0).reshape(25, 80, 160)).astype(np.float16)

    g2 = so3basisgrid.reshape(27, S).astype(np.float32)      # [mln, l]
    g2t16 = g2.T.astype(np.float16)                          # [l, mln]

    # A-tile so3 lhsT: ga[p, mt, el2*27+mln]; p = e*10+l (only p < 128 rows
    # live in the A conv tile). mt covers e in [4mt, 4mt+4).
    ga = np.zeros((128, 4, 108), np.float16)
    for mt in range(4):
        for el2 in range(4):
            e = 4 * mt + el2
            for l in range(S):
                p = e * S + l
                if p < 128:
                    ga[p, mt, el2 * 27:(el2 + 1) * 27] = g2t16[l]
    # B-tile so3 lhsT (only mt=3, e 12..15), replicated per chunk slot:
    # B row r: r=0,1 -> (e12, l8+r); r=2+10*m+l -> (e13+m, l)
    gb = np.zeros((128, 108), np.float16)
    for cslot in range(4):
        for r in range(32):
            if r < 2:
                e, l = 12, 8 + r
            else:
                e, l = 13 + (r - 2) // S, (r - 2) % S
            el2 = e - 12
            gb[32 * cslot + r, el2 * 27:(el2 + 1) * 27] = g2t16[l]

    # weighted-moment lhsT: wnd[(el2*27+mln), mt*16+e], e = 4mt+el2
    w_flat = np.asarray(w_i, np.float32)[(np.arange(27) // 3) % 3]
    wnd = np.zeros((108, 4, 16), np.float16)
    for mt in range(4):
        for el2 in range(4):
            e = 4 * mt + el2
            wnd[el2 * 27:(el2 + 1) * 27, mt, e] = w_flat.astype(np.float16)
    wnd = wnd.reshape(108, 64)

    bias_arr = np.asarray(bias, np.float32).reshape(16, 1)

    in_maps = []
    for c in range(NCORES):
        b, q = divmod(c, 4)
        slab = x[b, :, q * SLAB:q * SLAB + SLAB_IN]          # (16, 13, 40, 40)
        r_arr = np.empty((5, 16, SLAB_IN, D_OUT, D_IN), np.float16)
        for j in range(5):
            r_arr[j] = slab[:, :, j:j + D_OUT, :]
        in_maps.append({
            "r": np.ascontiguousarray(r_arr.reshape(80, SLAB_IN, D_OUT, D_IN)),
            "w": w_arr,
            "ga": np.ascontiguousarray(ga),
            "gb": np.ascontiguousarray(gb),
            "wnd": np.ascontiguousarray(wnd),
            "bias": bias_arr,
        })
    return in_maps


def _run(inputs, trace=False, **run_kwargs):
    inputs = {k: np.asarray(v) for k, v in inputs.items()}
    in_maps = _host_prep(**inputs)
    if "nc" not in _prog_cache:
        _prog_cache["nc"] = _build_program()
    nc = _prog_cache["nc"]
    try:
        res = run_bass_kernel_spmd(nc, in_maps, core_ids=list(range(NCORES)),
                                   trace=trace, **run_kwargs)
    except ModuleNotFoundError as e:
        if "axon_hooks" not in str(e):
            raise
        # Tracing requested (e.g. BASS_TRACE=1) but this axon client has no
        # NTFF profile hook — rerun with tracing disabled.
        os.environ["BASS_NEVER_TRACE"] = "1"
        res = run_bass_kernel_spmd(nc, in_maps, core_ids=list(range(NCORES)),
                                   trace=False, **run_kwargs)
    out = np.empty((B, COUT, D_OUT, D_OUT, D_OUT), np.float32)
    for c in range(NCORES):
        b, q = divmod(c, 4)
        out[b, :, q * SLAB:(q + 1) * SLAB] = res.results[c]["y"]
    return out, res


def kernel(**inputs):
    out, _ = _run(inputs)
    return out


# revision 15
# speedup vs baseline: 1.0003x; 1.0003x over previous
"""Trainium2 Bass kernel for nn_InvLocalPatOrientConvolution.

Computation:
  1. Host: synthesize the 160-channel 5x5x5 conv filter from
     weight/zeroweight/basis_functions/wigner indices (3.2 MFLOP) and lay out
     per-core operands (fp16).
  2. Device (8 NeuronCores, SPMD): VALID 3D conv as PE matmuls (contraction =
     5 y-taps x 16 input channels = 80 partitions; x/z tap shifts expressed as
     AP offsets into a 5x-replicated SBUF-resident input) + SO(3) grid pooling
     (relu-weighted second-moment ratio) also on the PE.
     Channel split: 128-channel A-tile (full array) + 32-channel B-tile packed
     4 chunks at a time into the 4 PE column groups via tile_position.
     Sharding: batch (2) x output-X-slabs (4) -> 8 cores.
  3. Host: gather per-core slabs into the full (2,16,36,36,36) output.
"""

import os
import sys

for _p in ("/root/.axon_site/_ro/trn_rl_repo", "/opt/trn_rl_repo"):
    if os.path.isdir(_p) and _p not in sys.path:
        sys.path.insert(0, _p)

import numpy as np

import concourse.mybir as mybir
from concourse import bacc
from concourse.tile import TileContext
from concourse.bass_utils import run_bass_kernel_spmd

# Problem constants (hardcoded per harness contract)
ORDER = 2
KS = 5            # conv kernel size
CIN = 16
COUT = 16
EPS = 1e-16
S = 10            # wigner rows
B = 2
D_IN = 40         # input spatial
D_OUT = 36        # output spatial
SLAB = 9          # output X planes per core (36/4)
SLAB_IN = SLAB + KS - 1   # 13 input X planes per core
NCORES = 8
YB = 12           # y-block per chunk
NCHUNK = YB * D_OUT       # 432 columns per matmul chunk

F16 = mybir.dt.float16
F32 = mybir.dt.float32

_prog_cache = {}


def _build_program(repeat=1):
    """Build the SPMD device program (identical on all 8 cores)."""
    nc = bacc.Bacc("TRN2")

    r_d = nc.dram_tensor("r", [80, SLAB_IN, D_OUT, D_IN], F16, kind="ExternalInput")
    w_d = nc.dram_tensor("w", [25, 80, 160], F16, kind="ExternalInput")
    ga_d = nc.dram_tensor("ga", [128, 4, 108], F16, kind="ExternalInput")
    gb_d = nc.dram_tensor("gb", [128, 108], F16, kind="ExternalInput")
    wnd_d = nc.dram_tensor("wnd", [108, 64], F16, kind="ExternalInput")
    bias_d = nc.dram_tensor("bias", [16, 1], F32, kind="ExternalInput")
    y_d = nc.dram_tensor("y", [16, SLAB, D_OUT, D_OUT], F32, kind="ExternalOutput")

    chunks = [(xr, cy) for xr in range(SLAB) for cy in range(3)]
    groups = [chunks[i:i + 4] for i in range(0, len(chunks), 4)]

    with TileContext(nc) as tc:
        with tc.tile_pool(name="const", bufs=1) as cpool, \
             tc.tile_pool(name="work", bufs=3) as wpool, \
             tc.tile_pool(name="casb", bufs=9) as capool, \
             tc.tile_pool(name="rrel", bufs=5) as rpool, \
             tc.tile_pool(name="conv_ps", bufs=3, space="PSUM") as conv_pool, \
             tc.tile_pool(name="convb_ps", bufs=1, space="PSUM") as convb_pool, \
             tc.tile_pool(name="a_ps", bufs=3, space="PSUM") as a_pool, \
             tc.tile_pool(name="nd_ps", bufs=1, space="PSUM") as nd_pool:

            # ---- resident constants (weights first so chunk 0 can start
            # as soon as its 5 input planes land) ----
            wt = cpool.tile([80, 25, 160], F16, tag="wt2")
            for ik in range(25):
                nc.sync.dma_start(out=wt[:, ik, :], in_=w_d[ik, :, :])
            gat = cpool.tile([128, 4, 108], F16)
            gbt = cpool.tile([128, 108], F16)
            wndt = cpool.tile([108, 64], F16)
            biast = cpool.tile([16, 1], F32)
            nc.sync.dma_start(out=gat[:], in_=ga_d[:])
            nc.sync.dma_start(out=gbt[:], in_=gb_d[:])
            nc.sync.dma_start(out=wndt[:], in_=wnd_d[:])
            nc.sync.dma_start(out=biast[:], in_=bias_d[:])
            rts = []
            for p in range(SLAB_IN):
                rt = cpool.tile([80, D_OUT, D_IN], F16, tag=f"rt{p}")
                nc.sync.dma_start(out=rt[:], in_=r_d[:, p, :, :])
                rts.append(rt)

            for _rep in range(repeat):
              for grp in groups:
                # ---- conv A-tiles (128 channels, full array) ----
                ca_sbs = []
                for (xr, cy) in grp:
                    y0 = cy * YB
                    cps = conv_pool.tile([128, NCHUNK], F32, tag="cps")
                    t = 0
                    for i in range(KS):
                        for k in range(KS):
                            rhs = rts[xr + i][:, y0:y0 + YB, k:k + D_OUT]
                            lhsT = wt[:, i * KS + k, 0:128]
                            nc.tensor.matmul(cps[:], lhsT, rhs,
                                             start=(t == 0), stop=(t == 24))
                            t += 1
                    ca = capool.tile([128, NCHUNK], F16, tag="ca")
                    nc.scalar.copy(ca[:], cps[:])
                    ca_sbs.append(ca)

                # ---- conv B-tile (32 channels) col-tiled over the group ----
                cbps = convb_pool.tile([128, NCHUNK], F32, tag="cbps")
                for t, (i, k) in enumerate((i, k) for i in range(KS)
                                           for k in range(KS)):
                    lhsT = wt[:, i * KS + k, 128:160]
                    for c, (xr, cy) in enumerate(grp):
                        y0 = cy * YB
                        rhs = rts[xr + i][:, y0:y0 + YB, k:k + D_OUT]
                        nc.tensor.matmul(
                            cbps[32 * c:32 * (c + 1), :], lhsT, rhs,
                            start=(t == 0), stop=(t == 24),
                            tile_position=(0, 32 * c),
                        )
                cb = capool.tile([128, NCHUNK], F16, tag="cb")
                nc.scalar.copy(cb[:], cbps[:])

                # ---- so3 pooling per chunk ----
                for c, (xr, cy) in enumerate(grp):
                    y0 = cy * YB
                    # num/den partial sums packed into the 4 PE column groups
                    # of ONE psum tile: rows 0-15 / 32-47 = num (mt even/odd),
                    # rows 64-79 / 96-111 = den (mt even/odd).
                    nd_ps = nd_pool.tile([128, NCHUNK], F32, tag="nd")
                    rrels, r2s = [], []
                    for mt in range(4):
                        aps = a_pool.tile([108, NCHUNK], F32, tag="aps")
                        last = (mt == 3)
                        nc.tensor.matmul(aps[:], gat[:, mt, :], ca_sbs[c][:],
                                         start=True, stop=not last)
                        if last:
                            nc.tensor.matmul(
                                aps[:],
                                gbt[32 * c:32 * (c + 1), :],
                                cb[32 * c:32 * (c + 1), :],
                                start=False, stop=True,
                                tile_position=(32 * c, 0),
                            )
                        rrel = rpool.tile([108, NCHUNK], F16, tag="rrel")
                        nc.scalar.activation(rrel[:], aps[:],
                                             mybir.ActivationFunctionType.Relu)
                        r2 = rpool.tile([108, NCHUNK], F16, tag="r2")
                        nc.vector.tensor_mul(r2[:], rrel[:], rrel[:])
                        rrels.append(rrel)
                        r2s.append(r2)
                    # all 8 partial-moment matmuls back-to-back so the 4 PE
                    # column groups stream them concurrently (span ~2N)
                    for mt in range(4):
                        wnd_g = wndt[:, mt * 16:(mt + 1) * 16]
                        cg = 32 * (mt % 2)
                        nc.tensor.matmul(nd_ps[cg:cg + 16, :], wnd_g,
                                         r2s[mt][:],
                                         start=(mt < 2), stop=(mt >= 2),
                                         tile_position=(0, cg))
                        nc.tensor.matmul(nd_ps[64 + cg:64 + cg + 16, :],
                                         wnd_g, rrels[mt][:],
                                         start=(mt < 2), stop=(mt >= 2),
                                         tile_position=(0, 64 + cg))

                    num_a = wpool.tile([16, NCHUNK], F32, tag="num_a")
                    nc.scalar.copy(num_a[:], nd_ps[0:16, :])
                    den_a = wpool.tile([16, NCHUNK], F32, tag="den_a")
                    nc.scalar.activation(den_a[:], nd_ps[64:80, :],
                                         mybir.ActivationFunctionType.Copy,
                                         bias=EPS)
                    num_sb = wpool.tile([16, NCHUNK], F32, tag="num_sb")
                    nc.vector.tensor_add(num_sb[:], num_a[:], nd_ps[32:48, :])
                    den_sb = wpool.tile([16, NCHUNK], F32, tag="den_sb")
                    nc.vector.tensor_add(den_sb[:], den_a[:], nd_ps[96:112, :])
                    recip = wpool.tile([16, NCHUNK], F32, tag="recip")
                    nc.vector.reciprocal(recip[:], den_sb[:])
                    out_sb = wpool.tile([16, NCHUNK], F32, tag="out_sb")
                    nc.vector.tensor_mul(out_sb[:], num_sb[:], recip[:])
                    nc.vector.tensor_scalar_add(out_sb[:], out_sb[:],
                                                biast[:, 0:1])

                    dst = y_d[:, xr, y0:y0 + YB, :]
                    nc.sync.dma_start(out=dst, in_=out_sb[:].rearrange(
                        "p (a b) -> p a b", a=YB))

    nc.finalize()
    return nc


def _synthesize_filter(weight, zeroweight, basis_functions, wig_w, wig_b):
    """Replicate the reference's kernel synthesis in fp32 numpy.

    Returns kern6[l, e, d, i, j, k] of shape (10, 16, 16, 5, 5, 5)."""
    zero_ext = np.concatenate(
        [zeroweight[None, None],
         np.zeros((ORDER ** 2 - 1, 1, CIN, COUT), weight.dtype)], axis=0)
    wfull = np.concatenate([zero_ext, weight], axis=1)       # (4, 10, 16, 16)
    wg = wfull[wig_w]                                        # (10, 10, 16, 16)
    bg = basis_functions[wig_b]                              # (10, 10, 5, 5, 5)
    kern6 = np.einsum("lred,lrijk->ledijk", wg, bg)          # (10,16,16,5,5,5)
    return np.ascontiguousarray(kern6.astype(np.float32))


def _host_prep(x, weight, zeroweight, bias, so3basisgrid, w_i,
               basis_functions, wig_w, wig_b):
    kern6 = _synthesize_filter(weight, zeroweight, basis_functions, wig_w, wig_b)

    # conv weights: W[i*5+k, j*16+d, e*10+l]  (cols e-major; A = cols 0..127)
    w_arr = np.ascontiguousarray(
        kern6.transpose(3, 5, 4, 2, 1, 0).reshape(25, 80, 160)).astype(np.float16)

    g2 = so3basisgrid.reshape(27, S).astype(np.float32)      # [mln, l]
    g2t16 = g2.T.astype(np.float16)                          # [l, mln]

    # A-tile so3 lhsT: ga[p, mt, el2*27+mln]; p = e*10+l (only p < 128 rows
    # live in the A conv tile). mt covers e in [4mt, 4mt+4).
    ga = np.zeros((128, 4, 108), np.float16)
    for mt in range(4):
        for el2 in range(4):
            e = 4 * mt + el2
            for l in range(S):
                p = e * S + l
                if p < 128:
                    ga[p, mt, el2 * 27:(el2 + 1) * 27] = g2t16[l]
    # B-tile so3 lhsT (only mt=3, e 12..15), replicated per chunk slot:
    # B row r: r=0,1 -> (e12, l8+r); r=2+10*m+l -> (e13+m, l)
    gb = np.zeros((128, 108), np.float16)
    for cslot in range(4):
        for r in range(32):
            if r < 2:
                e, l = 12, 8 + r
            else:
                e, l = 13 + (r - 2) // S, (r - 2) % S
            el2 = e - 12
            gb[32 * cslot + r, el2 * 27:(el2 + 1) * 27] = g2t16[l]

    # weighted-moment lhsT: wnd[(el2*27+mln), mt*16+e], e = 4mt+el2
    w_flat = np.asarray(w_i, np.float32)[(np.arange(27) // 3) % 3]
    wnd = np.zeros((108, 4, 16), np.float16)
    for mt in range(4):
        for el2 in range(4):
            e = 4 * mt + el2
            wnd[el2 * 27:(el2 + 1) * 27, mt, e] = w_flat.astype(np.float16)
    wnd = wnd.reshape(108, 64)

    bias_arr = np.asarray(bias, np.float32).reshape(16, 1)

    in_maps = []
    for c in range(NCORES):
        b, q = divmod(c, 4)
        slab = x[b, :, q * SLAB:q * SLAB + SLAB_IN]          # (16, 13, 40, 40)
        r_arr = np.empty((5, 16, SLAB_IN, D_OUT, D_IN), np.float16)
        for j in range(5):
            r_arr[j] = slab[:, :, j:j + D_OUT, :]
        in_maps.append({
            "r": np.ascontiguousarray(r_arr.reshape(80, SLAB_IN, D_OUT, D_IN)),
            "w": w_arr,
            "ga": np.ascontiguousarray(ga),
            "gb": np.ascontiguousarray(gb),
            "wnd": np.ascontiguousarray(wnd),
            "bias": bias_arr,
        })
    return in_maps


def _run(inputs, trace=False, **run_kwargs):
    inputs = {k: np.asarray(v) for k, v in inputs.items()}
    in_maps = _host_prep(**inputs)
    if "nc" not in _prog_cache:
        _prog_cache["nc"] = _build_program()
    nc = _prog_cache["nc"]
    try:
        res = run_bass_kernel_spmd(nc, in_maps, core_ids=list(range(NCORES)),
                                   trace=trace, **run_kwargs)
    except ModuleNotFoundError as e:
        if "axon_hooks" not in str(e):
            raise
        # Tracing requested (e.g. BASS_TRACE=1) but this axon client has no
        # NTFF profile hook — rerun with tracing disabled.
        os.environ["BASS_NEVER_TRACE"] = "1"
        res = run_bass_kernel_spmd(nc, in_maps, core_ids=list(range(NCORES)),
                                   trace=False, **run_kwargs)
    out = np.empty((B, COUT, D_OUT, D_OUT, D_OUT), np.float32)
    for c in range(NCORES):
        b, q = divmod(c, 4)
        out[b, :, q * SLAB:(q + 1) * SLAB] = res.results[c]["y"]
    return out, res


def kernel(**inputs):
    out, _ = _run(inputs)
    return out


# revision 17
# speedup vs baseline: 1.0500x; 1.0497x over previous
"""Trainium2 Bass kernel for nn_InvLocalPatOrientConvolution.

Computation:
  1. Host: synthesize the 160-channel 5x5x5 conv filter from
     weight/zeroweight/basis_functions/wigner indices (3.2 MFLOP) and lay out
     per-core operands (fp16).
  2. Device (8 NeuronCores, SPMD): VALID 3D conv as PE matmuls (contraction =
     5 y-taps x 16 input channels = 80 partitions; x/z tap shifts expressed as
     AP offsets into a 5x-replicated SBUF-resident input) + SO(3) grid pooling
     (relu-weighted second-moment ratio) also on the PE.
     Channel split: 128-channel A-tile (full array) + 32-channel B-tile packed
     4 chunks at a time into the 4 PE column groups via tile_position.
     Sharding: batch (2) x output-X-slabs (4) -> 8 cores.
  3. Host: gather per-core slabs into the full (2,16,36,36,36) output.
"""

import os
import sys

for _p in ("/root/.axon_site/_ro/trn_rl_repo", "/opt/trn_rl_repo"):
    if os.path.isdir(_p) and _p not in sys.path:
        sys.path.insert(0, _p)

import numpy as np

import concourse.mybir as mybir
from concourse import bacc
from concourse.tile import TileContext
from concourse.bass_utils import run_bass_kernel_spmd

# Problem constants (hardcoded per harness contract)
ORDER = 2
KS = 5            # conv kernel size
CIN = 16
COUT = 16
EPS = 1e-16
S = 10            # wigner rows
B = 2
D_IN = 40         # input spatial
D_OUT = 36        # output spatial
SLAB = 9          # output X planes per core (36/4)
SLAB_IN = SLAB + KS - 1   # 13 input X planes per core
NCORES = 8
YB = 12           # y-block per chunk
NCHUNK = YB * D_OUT       # 432 columns per matmul chunk

F16 = mybir.dt.float16
F32 = mybir.dt.float32

_prog_cache = {}


def _build_program(repeat=1):
    """Build the SPMD device program (identical on all 8 cores)."""
    nc = bacc.Bacc("TRN2")

    r_d = nc.dram_tensor("r", [SLAB_IN, 80, D_OUT, D_IN], F16, kind="ExternalInput")
    w_d = nc.dram_tensor("w", [25, 80, 160], F16, kind="ExternalInput")
    ga_d = nc.dram_tensor("ga", [128, 4, 108], F16, kind="ExternalInput")
    gb_d = nc.dram_tensor("gb", [128, 108], F16, kind="ExternalInput")
    wnd_d = nc.dram_tensor("wnd", [108, 64], F16, kind="ExternalInput")
    bias_d = nc.dram_tensor("bias", [16, 1], F32, kind="ExternalInput")
    y_d = nc.dram_tensor("y", [16, SLAB, D_OUT, D_OUT], F32, kind="ExternalOutput")

    chunks = [(xr, cy) for xr in range(SLAB) for cy in range(3)]
    groups = [chunks[i:i + 4] for i in range(0, len(chunks), 4)]

    with TileContext(nc) as tc:
        with tc.tile_pool(name="const", bufs=1) as cpool, \
             tc.tile_pool(name="work", bufs=3) as wpool, \
             tc.tile_pool(name="casb", bufs=9) as capool, \
             tc.tile_pool(name="rrel", bufs=5) as rpool, \
             tc.tile_pool(name="conv_ps", bufs=3, space="PSUM") as conv_pool, \
             tc.tile_pool(name="convb_ps", bufs=1, space="PSUM") as convb_pool, \
             tc.tile_pool(name="a_ps", bufs=3, space="PSUM") as a_pool, \
             tc.tile_pool(name="nd_ps", bufs=1, space="PSUM") as nd_pool:

            # ---- resident constants. Order: first 5 input planes + the
            # conv weights (what chunk 0 needs), then the rest — cuts the
            # PE startup stall. Each plane DMA is a contiguous 230KB read.
            rts = []
            for p in range(SLAB_IN):
                rt = cpool.tile([80, D_OUT, D_IN], F16, tag=f"rt{p}")
                rts.append(rt)
            for p in range(KS):
                nc.sync.dma_start(out=rts[p][:], in_=r_d[p, :, :, :])
            wt = cpool.tile([80, 25, 160], F16, tag="wt2")
            for ik in range(25):
                nc.sync.dma_start(out=wt[:, ik, :], in_=w_d[ik, :, :])
            gat = cpool.tile([128, 4, 108], F16)
            gbt = cpool.tile([128, 108], F16)
            wndt = cpool.tile([108, 64], F16)
            biast = cpool.tile([16, 1], F32)
            nc.sync.dma_start(out=gat[:], in_=ga_d[:])
            nc.sync.dma_start(out=gbt[:], in_=gb_d[:])
            nc.sync.dma_start(out=wndt[:], in_=wnd_d[:])
            nc.sync.dma_start(out=biast[:], in_=bias_d[:])
            for p in range(KS, SLAB_IN):
                nc.sync.dma_start(out=rts[p][:], in_=r_d[p, :, :, :])

            for _rep in range(repeat):
              for grp in groups:
                # ---- conv A-tiles (128 channels, full array) ----
                ca_sbs = []
                for (xr, cy) in grp:
                    y0 = cy * YB
                    cps = conv_pool.tile([128, NCHUNK], F32, tag="cps")
                    t = 0
                    for i in range(KS):
                        for k in range(KS):
                            rhs = rts[xr + i][:, y0:y0 + YB, k:k + D_OUT]
                            lhsT = wt[:, i * KS + k, 0:128]
                            nc.tensor.matmul(cps[:], lhsT, rhs,
                                             start=(t == 0), stop=(t == 24))
                            t += 1
                    ca = capool.tile([128, NCHUNK], F16, tag="ca")
                    nc.scalar.copy(ca[:], cps[:])
                    ca_sbs.append(ca)

                # ---- conv B-tile (32 channels) col-tiled over the group ----
                cbps = convb_pool.tile([128, NCHUNK], F32, tag="cbps")
                for t, (i, k) in enumerate((i, k) for i in range(KS)
                                           for k in range(KS)):
                    lhsT = wt[:, i * KS + k, 128:160]
                    for c, (xr, cy) in enumerate(grp):
                        y0 = cy * YB
                        rhs = rts[xr + i][:, y0:y0 + YB, k:k + D_OUT]
                        nc.tensor.matmul(
                            cbps[32 * c:32 * (c + 1), :], lhsT, rhs,
                            start=(t == 0), stop=(t == 24),
                            tile_position=(0, 32 * c),
                        )
                cb = capool.tile([128, NCHUNK], F16, tag="cb")
                nc.scalar.copy(cb[:], cbps[:])

                # ---- so3 pooling per chunk ----
                for c, (xr, cy) in enumerate(grp):
                    y0 = cy * YB
                    # num/den partial sums packed into the 4 PE column groups
                    # of ONE psum tile: rows 0-15 / 32-47 = num (mt even/odd),
                    # rows 64-79 / 96-111 = den (mt even/odd).
                    nd_ps = nd_pool.tile([128, NCHUNK], F32, tag="nd")
                    rrels, r2s = [], []
                    for mt in range(4):
                        aps = a_pool.tile([108, NCHUNK], F32, tag="aps")
                        last = (mt == 3)
                        nc.tensor.matmul(aps[:], gat[:, mt, :], ca_sbs[c][:],
                                         start=True, stop=not last)
                        if last:
                            nc.tensor.matmul(
                                aps[:],
                                gbt[32 * c:32 * (c + 1), :],
                                cb[32 * c:32 * (c + 1), :],
                                start=False, stop=True,
                                tile_position=(32 * c, 0),
                            )
                        rrel = rpool.tile([108, NCHUNK], F16, tag="rrel")
                        nc.scalar.activation(rrel[:], aps[:],
                                             mybir.ActivationFunctionType.Relu)
                        r2 = rpool.tile([108, NCHUNK], F16, tag="r2")
                        nc.vector.tensor_mul(r2[:], rrel[:], rrel[:])
                        rrels.append(rrel)
                        r2s.append(r2)
                    # all 8 partial-moment matmuls back-to-back so the 4 PE
                    # column groups stream them concurrently (span ~2N)
                    for mt in range(4):
                        wnd_g = wndt[:, mt * 16:(mt + 1) * 16]
                        cg = 32 * (mt % 2)
                        nc.tensor.matmul(nd_ps[cg:cg + 16, :], wnd_g,
                                         r2s[mt][:],
                                         start=(mt < 2), stop=(mt >= 2),
                                         tile_position=(0, cg))
                        nc.tensor.matmul(nd_ps[64 + cg:64 + cg + 16, :],
                                         wnd_g, rrels[mt][:],
                                         start=(mt < 2), stop=(mt >= 2),
                                         tile_position=(0, 64 + cg))

                    num_a = wpool.tile([16, NCHUNK], F32, tag="num_a")
                    nc.scalar.copy(num_a[:], nd_ps[0:16, :])
                    den_a = wpool.tile([16, NCHUNK], F32, tag="den_a")
                    nc.scalar.activation(den_a[:], nd_ps[64:80, :],
                                         mybir.ActivationFunctionType.Copy,
                                         bias=EPS)
                    num_sb = wpool.tile([16, NCHUNK], F32, tag="num_sb")
                    nc.vector.tensor_add(num_sb[:], num_a[:], nd_ps[32:48, :])
                    den_sb = wpool.tile([16, NCHUNK], F32, tag="den_sb")
                    nc.vector.tensor_add(den_sb[:], den_a[:], nd_ps[96:112, :])
                    recip = wpool.tile([16, NCHUNK], F32, tag="recip")
                    nc.vector.reciprocal(recip[:], den_sb[:])
                    out_sb = wpool.tile([16, NCHUNK], F32, tag="out_sb")
                    nc.vector.tensor_mul(out_sb[:], num_sb[:], recip[:])
                    nc.vector.tensor_scalar_add(out_sb[:], out_sb[:],
                                                biast[:, 0:1])

                    dst = y_d[:, xr, y0:y0 + YB, :]
                    nc.sync.dma_start(out=dst, in_=out_sb[:].rearrange(
                        "p (a b) -> p a b", a=YB))

    nc.finalize()
    return nc


def _synthesize_filter(weight, zeroweight, basis_functions, wig_w, wig_b):
    """Replicate the reference's kernel synthesis in fp32 numpy.

    Returns kern6[l, e, d, i, j, k] of shape (10, 16, 16, 5, 5, 5)."""
    zero_ext = np.concatenate(
        [zeroweight[None, None],
         np.zeros((ORDER ** 2 - 1, 1, CIN, COUT), weight.dtype)], axis=0)
    wfull = np.concatenate([zero_ext, weight], axis=1)       # (4, 10, 16, 16)
    wg = wfull[wig_w]                                        # (10, 10, 16, 16)
    bg = basis_functions[wig_b]                              # (10, 10, 5, 5, 5)
    kern6 = np.einsum("lred,lrijk->ledijk", wg, bg)          # (10,16,16,5,5,5)
    return np.ascontiguousarray(kern6.astype(np.float32))


def _host_prep(x, weight, zeroweight, bias, so3basisgrid, w_i,
               basis_functions, wig_w, wig_b):
    kern6 = _synthesize_filter(weight, zeroweight, basis_functions, wig_w, wig_b)

    # conv weights: W[i*5+k, j*16+d, e*10+l]  (cols e-major; A = cols 0..127)
    w_arr = np.ascontiguousarray(
        kern6.transpose(3, 5, 4, 2, 1, 0).reshape(25, 80, 160)).astype(np.float16)

    g2 = so3basisgrid.reshape(27, S).astype(np.float32)      # [mln, l]
    g2t16 = g2.T.astype(np.float16)                          # [l, mln]

    # A-tile so3 lhsT: ga[p, mt, el2*27+mln]; p = e*10+l (only p < 128 rows
    # live in the A conv tile). mt covers e in [4mt, 4mt+4).
    ga = np.zeros((128, 4, 108), np.float16)
    for mt in range(4):
        for el2 in range(4):
            e = 4 * mt + el2
            for l in range(S):
                p = e * S + l
                if p < 128:
                    ga[p, mt, el2 * 27:(el2 + 1) * 27] = g2t16[l]
    # B-tile so3 lhsT (only mt=3, e 12..15), replicated per chunk slot:
    # B row r: r=0,1 -> (e12, l8+r); r=2+10*m+l -> (e13+m, l)
    gb = np.zeros((128, 108), np.float16)
    for cslot in range(4):
        for r in range(32):
            if r < 2:
                e, l = 12, 8 + r
            else:
                e, l = 13 + (r - 2) // S, (r - 2) % S
            el2 = e - 12
            gb[32 * cslot + r, el2 * 27:(el2 + 1) * 27] = g2t16[l]

    # weighted-moment lhsT: wnd[(el2*27+mln), mt*16+e], e = 4mt+el2
    w_flat = np.asarray(w_i, np.float32)[(np.arange(27) // 3) % 3]
    wnd = np.zeros((108, 4, 16), np.float16)
    for mt in range(4):
        for el2 in range(4):
            e = 4 * mt + el2
            wnd[el2 * 27:(el2 + 1) * 27, mt, e] = w_flat.astype(np.float16)
    wnd = wnd.reshape(108, 64)

    bias_arr = np.asarray(bias, np.float32).reshape(16, 1)

    in_maps = []
    for c in range(NCORES):
        b, q = divmod(c, 4)
        slab = x[b, :, q * SLAB:q * SLAB + SLAB_IN]          # (16, 13, 40, 40)
        r_arr = np.empty((SLAB_IN, 5, 16, D_OUT, D_IN), np.float16)
        for j in range(5):
            r_arr[:, j] = slab[:, :, j:j + D_OUT, :].transpose(1, 0, 2, 3)
        in_maps.append({
            "r": np.ascontiguousarray(r_arr.reshape(SLAB_IN, 80, D_OUT, D_IN)),
            "w": w_arr,
            "ga": np.ascontiguousarray(ga),
            "gb": np.ascontiguousarray(gb),
            "wnd": np.ascontiguousarray(wnd),
            "bias": bias_arr,
        })
    return in_maps


def _run(inputs, trace=False, **run_kwargs):
    inputs = {k: np.asarray(v) for k, v in inputs.items()}
    in_maps = _host_prep(**inputs)
    if "nc" not in _prog_cache:
        _prog_cache["nc"] = _build_program()
    nc = _prog_cache["nc"]
    try:
        res = run_bass_kernel_spmd(nc, in_maps, core_ids=list(range(NCORES)),
                                   trace=trace, **run_kwargs)
    except ModuleNotFoundError as e:
        if "axon_hooks" not in str(e):
            raise
        # Tracing requested (e.g. BASS_TRACE=1) but this axon client has no
        # NTFF profile hook — rerun with tracing disabled.
        os.environ["BASS_NEVER_TRACE"] = "1"
        res = run_bass_kernel_spmd(nc, in_maps, core_ids=list(range(NCORES)),
                                   trace=False, **run_kwargs)
    out = np.empty((B, COUT, D_OUT, D_OUT, D_OUT), np.float32)
    for c in range(NCORES):
        b, q = divmod(c, 4)
        out[b, :, q * SLAB:(q + 1) * SLAB] = res.results[c]["y"]
    return out, res


def kernel(**inputs):
    out, _ = _run(inputs)
    return out
